# revision 21
# baseline (speedup 1.0000x reference)
"""Trainium2 Bass kernel for nn_CFHoTWrapper (fiber-adapter + causal EMA + gate).

Contract: kernel(**inputs) takes the FULL unsharded inputs (hidden [8,8192,2048],
adapter weights) and returns (gate [8,8192], field [8,8192], risk scalar),
matching reference.reference(). Work is data-parallel over batch B across the
8 NeuronCores (one batch element per core); the tiny adapter weights are
replicated.

Math notes:
  - fiber = h @ fiber_w.T is linear and feeds straight into the concat+w1
    matmul, so it folds:  combined @ w1.T = h @ (w1[:, :D] + w1[:, D:] @ fiber_w).T
    Host precomputes Weff [C, D] once.
  - The causal EMA over S is a linear recurrence; it is computed as a blocked
    prefix: a [128,128] lower-triangular matmul gives within-block prefixes,
    a [NB,NB] triangular matmul gives cross-block carries, and a rank-1
    outer-product matmul (alpha^(l+1) x carry_j) accumulates the carry terms.
"""

import os
import sys

for _p in ("/opt/trn_rl_repo", "/root/.axon_site/_ro/trn_rl_repo"):
    if os.path.isdir(_p) and _p not in sys.path:
        sys.path.insert(0, _p)

import numpy as np

import concourse.bass as bass
import concourse.mybir as mybir
import concourse.tile as tile
from concourse import bacc
from concourse.tile_rust import add_dep_helper
from concourse.bass_utils import run_bass_kernel_spmd

F32 = mybir.dt.float32
F16 = mybir.dt.float16
F32R = mybir.dt.float32r
AF = mybir.ActivationFunctionType

ALPHA = 0.9
P = 128
N_CORES = 8


def build_nc(S=8192, D=2048, C=64, CHUNK=512, b2val=0.0, lam=1.0, use_f32r=True,
             act="erf"):
    """Build + compile the per-core Bass program (SPMD: identical on all cores).

    Inputs (per core): hidden_b [S, D], weffT [D, C], w2col [C, 1], b1col [C, 1],
    ident [128, 128], tlt [128, 128], t2r [NB, NB], apow [1, 128], ones1 [1, 1].
    Outputs: gate_out [NB, 128], field_out [NB, 128] (row-major == natural S
    order), rsum_out [128, 1] (partial sums of delta for the risk mean).
    """
    NB = S // P           # EMA blocks of 128 tokens
    NCH = S // CHUNK      # token chunks processed per loop iteration
    SPC = CHUNK // P      # 128-token tiles per chunk
    KD = D // P           # contraction blocks
    assert S % CHUNK == 0 and CHUNK % P == 0 and D % P == 0

    nc = bacc.Bacc("TRN2", target_bir_lowering=False, debug=False,
                   enable_asserts=False)

    h_d = nc.dram_tensor("hidden_b", [S, D], F32, kind="ExternalInput").ap()
    weffT_d = nc.dram_tensor("weffT", [D, C], F32, kind="ExternalInput").ap()
    w2c_d = nc.dram_tensor("w2col", [C, 1], F32, kind="ExternalInput").ap()
    b1c_d = nc.dram_tensor("b1col", [C, 1], F32, kind="ExternalInput").ap()
    b1s_d = nc.dram_tensor("b1s", [C, 1], F32, kind="ExternalInput").ap()
    id_d = nc.dram_tensor("ident", [P, P], F32, kind="ExternalInput").ap()
    tlt_d = nc.dram_tensor("tlt", [P, P], F32, kind="ExternalInput").ap()
    t2r_d = nc.dram_tensor("t2r", [NB, NB], F32, kind="ExternalInput").ap()
    apow_d = nc.dram_tensor("apow", [1, P], F32, kind="ExternalInput").ap()
    t127_d = nc.dram_tensor("t127", [P, 1], F32, kind="ExternalInput").ap()
    b2c_d = nc.dram_tensor("b2col", [P, 1], F32, kind="ExternalInput").ap()
    one_d = nc.dram_tensor("ones1", [1, 1], F32, kind="ExternalInput").ap()

    gate_o = nc.dram_tensor("gate_out", [NB, P], F32, kind="ExternalOutput").ap()
    field_o = nc.dram_tensor("field_out", [NB, P], F32, kind="ExternalOutput").ap()
    rsum_o = nc.dram_tensor("rsum_out", [P, 1], F32, kind="ExternalOutput").ap()
    warm_o = nc.dram_tensor("warm_out", [1, 1], F32, kind="ExternalOutput").ap()

    RT = F16 if use_f32r else F32

    with tile.TileContext(nc) as tc:
        with (
            tc.tile_pool(name="consts", bufs=1) as cpool,
            tc.tile_pool(name="hin", bufs=4) as hpool,
            tc.tile_pool(name="hhalf", bufs=2) as hhpool,
            tc.tile_pool(name="ht", bufs=4) as htpool,
            tc.tile_pool(name="hid", bufs=2) as hidpool,
            tc.tile_pool(name="misc", bufs=1) as mpool,
            tc.tile_pool(name="pt", bufs=3, space=bass.MemorySpace.PSUM) as ptpool,
            tc.tile_pool(name="ph", bufs=2, space=bass.MemorySpace.PSUM) as phpool,
            tc.tile_pool(name="pd", bufs=1, space=bass.MemorySpace.PSUM) as pdpool,
            tc.tile_pool(name="ptail", bufs=2, space=bass.MemorySpace.PSUM) as tailpool,
        ):
            weffT = cpool.tile([P, KD, C], F32, tag="weffT")
            nc.sync.dma_start(weffT[:], weffT_d.rearrange("(kd p) c -> p kd c", p=P))
            # cast the stationary weights once on DVE: fp16 weights get
            # FWL (fast weight load) on the PE; fp32/f32r LDWEIGHTS serialize.
            weffT_r = cpool.tile([P, KD, C], RT, tag="weffT_r")
            nc.vector.tensor_copy(weffT_r[:], weffT[:])
            ident = cpool.tile([P, P], F32, tag="ident")
            nc.sync.dma_start(ident[:], id_d[:])
            identr = cpool.tile([P, P], RT, tag="identr")
            nc.vector.tensor_copy(identr[:], ident[:])
            identh = cpool.tile([P, P], F16, tag="identh")
            nc.vector.tensor_copy(identh[:], ident[:])
            tlt = cpool.tile([P, P], F32, tag="tlt")
            nc.sync.dma_start(tlt[:], tlt_d[:])
            t2r = cpool.tile([NB, NB], F32, tag="t2r")
            nc.sync.dma_start(t2r[:], t2r_d[:])
            w2c = cpool.tile([C, 1], F32, tag="w2c")
            nc.sync.dma_start(w2c[:], w2c_d[:])
            w2h = cpool.tile([C, 1], RT, tag="w2h")
            nc.vector.tensor_copy(w2h[:], w2c[:])
            b1c = cpool.tile([C, 1], F32, tag="b1c")
            nc.sync.dma_start(b1c[:], b1c_d[:])
            b1s = cpool.tile([C, 1], F32, tag="b1s")
            nc.sync.dma_start(b1s[:], b1s_d[:])
            apow = cpool.tile([1, P], F32, tag="apow")
            nc.sync.dma_start(apow[:], apow_d[:])
            t127 = cpool.tile([P, 1], F32, tag="t127")
            nc.sync.dma_start(t127[:], t127_d[:])
            b2c = cpool.tile([P, 1], F32, tag="b2c")
            nc.sync.dma_start(b2c[:], b2c_d[:])
            one1 = cpool.tile([1, 1], F32, tag="one1")
            nc.sync.dma_start(one1[:], one_d[:])

            # delta (pre-softplus) accumulates here across the whole sequence,
            # laid out [l, j] with token s = j*128 + l.
            pdelta = pdpool.tile([P, NB], F32, tag="pdelta")

            # HAM warmup: transpose-mode matmuls don't count as PE-busy for
            # the activity monitor, so without real-matmul pressure the PE
            # stays clock-gated at 1.2 GHz. A dense burst of normal matmuls
            # here (overlapping the first chunk's DMA) flips it to 2.4 GHz;
            # afterwards a real matmul lands every ~1us so it never
            # re-throttles (MID window needs ~3.4us of continuous idle).
            pwarm = ptpool.tile([P, CHUNK], F32, tag="pt")
            for _w in range(32):
                nwk = min(KD, CHUNK // C)
                nc.tensor.matmul(pwarm[:, 0:nwk * C], identr[:],
                                 weffT_r[:, 0:nwk, :],
                                 start=True, stop=True)
            warm_sb = mpool.tile([1, 1], F32, tag="warmsb")
            nc.vector.tensor_copy(warm_sb[:], pwarm[0:1, 0:1])
            nc.sync.dma_start(warm_o[:], warm_sb[:])

            h_view = h_d.rearrange("(ch sp p) d -> ch p sp d", sp=SPC, p=P)
            dma_chain = []
            for ch in range(NCH):
                htile = hpool.tile([P, SPC, D], F32, tag="hin")
                # Issue each chunk as two half-DMAs on the SP ring, chained so
                # at most 2 halves are in flight. The SDMA engines round-robin
                # across ALL in-flight DMAs, so unbounded prefetch makes chunk
                # 0 finish ~4x late (a ~40us pipeline-fill bubble). The chain
                # keeps completion order == processing order while the ring
                # stays continuously busy.
                half = SPC // 2
                for hf in range(2):
                    dins = nc.sync.dma_start(
                        htile[:, hf * half:(hf + 1) * half, :],
                        h_view[ch][:, hf * half:(hf + 1) * half, :])
                    dma_chain.append(dins.ins)
                    if len(dma_chain) > 2:
                        add_dep_helper(dma_chain[-1], dma_chain[-3],
                                       reason="cap in-flight input DMAs at 2")
                # cast h to fp16 once per s-tile: fp16 transposes stream at
                # 1 cyc/row (vs 2 for fp32) and h gets rounded to 11 mantissa
                # bits anyway by the fp32r matmul (12 bits) - accuracy cost is
                # ~1.2x, PE cost halves. Cast rides on DVE/ACT headroom.
                hhalf = hhpool.tile([P, SPC, D], F16, tag="hhalf")
                for sp in range(SPC):
                    if sp < SPC - 1:
                        nc.vector.tensor_copy(hhalf[:, sp, :], htile[:, sp, :])
                    else:
                        nc.scalar.copy(hhalf[:, sp, :], htile[:, sp, :])
                phid = phpool.tile([C, CHUNK], F32, tag="phid")
                # software-pipelined by one kd: the matmul for kd-1 is emitted
                # after the transposes for kd, so the PE never stalls on the
                # PSUM->SBUF copy latency.
                pend = []
                for kd in range(KD):
                    pt = ptpool.tile([P, CHUNK], F16, tag="pt")
                    for sp in range(SPC):
                        nc.tensor.transpose(
                            pt[:, sp * P:(sp + 1) * P],
                            hhalf[:, sp, kd * P:(kd + 1) * P],
                            identh[:],
                        )
                    ht = htpool.tile([P, CHUNK], RT, tag="ht")
                    # DVE is faster per copy than ACT - bias the split 10:6
                    if kd % 8 < 5:
                        nc.vector.tensor_copy(ht[:], pt[:])
                    else:
                        nc.scalar.copy(ht[:], pt[:])
                    pend.append((ht, kd))
                    if len(pend) > 2:
                        h0, k0 = pend.pop(0)
                        nc.tensor.matmul(
                            phid[:], weffT_r[:, k0, :], h0[:],
                            start=(k0 == 0), stop=False)
                for h0, k0 in pend:
                    nc.tensor.matmul(
                        phid[:], weffT_r[:, k0, :], h0[:],
                        start=(k0 == 0), stop=(k0 == KD - 1))
                # exact gelu: u*0.5*(1+erf(u/sqrt2)); the 0.5 is folded into
                # w2col on the host. b1c is pre-scaled by 1/sqrt2 for the erf
                # input (b1s), raw for the u copy (b1c).
                # CoreSim has no Erf; act="tanh" swaps it for sim validation.
                afn = AF.Erf if act == "erf" else AF.Tanh
                erf_t = hidpool.tile([C, CHUNK], F32, tag="erf_t")
                nc.scalar.activation(erf_t[:], phid[:], afn,
                                     scale=float(1.0 / np.sqrt(2.0)), bias=b1s[:])
                u_sb = hidpool.tile([C, CHUNK], F32, tag="u_sb")
                nc.scalar.activation(u_sb[:], phid[:], AF.Identity, bias=b1c[:])
                hidT = hidpool.tile([C, CHUNK], RT, tag="hidT")
                nc.vector.scalar_tensor_tensor(
                    hidT[:], in0=erf_t[:], scalar=1.0, in1=u_sb[:],
                    op0=mybir.AluOpType.add, op1=mybir.AluOpType.mult)
                for b in range(SPC):
                    j = ch * SPC + b
                    nc.tensor.matmul(
                        pdelta[:, j:j + 1],
                        hidT[:, b * P:(b + 1) * P],
                        w2h[:],
                        start=True,
                        stop=True,
                    )

            # delta = softplus(. + b2) = ln(1 + exp(. + b2)) — Softplus has no
            # loadable ACT table here, but Exp and Ln share one set.
            # accum_out on the Ln gives per-partition delta sums for risk.
            texp = mpool.tile([P, NB], F32, tag="texp")
            nc.scalar.activation(texp[:], pdelta[:], AF.Exp, bias=b2c[:])
            t1p = mpool.tile([P, NB], F32, tag="t1p")
            nc.vector.tensor_scalar_add(t1p[:], texp[:], 1.0)
            delta_sb = mpool.tile([P, NB], F32, tag="delta")
            rsum = mpool.tile([P, 1], F32, tag="rsum")
            nc.scalar.activation(delta_sb[:], t1p[:], AF.Ln, accum_out=rsum[:])
            nc.sync.dma_start(rsum_o[:], rsum[:])

            # EMA: within-block prefix via triangular matmul
            plocal = tailpool.tile([P, NB], F32, tag="tail")
            nc.tensor.matmul(plocal[:], tlt[:], delta_sb[:], start=True, stop=True)
            # last row of each block (PSUM can't be read at partition 127;
            # recompute it from delta_sb with the T's last row) -> [1, NB]
            pllr = tailpool.tile([1, NB], F32, tag="tail")
            nc.tensor.matmul(pllr[:], t127[:], delta_sb[:], start=True, stop=True)
            ll_row = mpool.tile([1, NB], F32, tag="llrow")
            nc.vector.tensor_copy(ll_row[:], pllr[:])
            pllc = tailpool.tile([NB, 1], F32, tag="tail")
            nc.tensor.matmul(pllc[:], ll_row[:], one1[:], start=True, stop=True)
            llc = mpool.tile([NB, 1], F32, tag="llc")
            nc.vector.tensor_copy(llc[:], pllc[:])
            # carries into each block: C[j] = sum_i (alpha^128)^(j-1-i) ll[i], i<=j-1
            pcr = tailpool.tile([1, NB], F32, tag="tail")
            nc.tensor.matmul(pcr[:], llc[:], t2r[:], start=True, stop=True)
            crow = mpool.tile([1, NB], F32, tag="crow")
            nc.vector.tensor_copy(crow[:], pcr[:])
            # field[l, j] = local[l, j] + alpha^(l+1) * C[j]
            nc.tensor.matmul(plocal[:], apow[:], crow[:], start=False, stop=True,
                             skip_group_check=True)
            field_sb = mpool.tile([P, NB], F32, tag="fieldsb")
            nc.vector.tensor_copy(field_sb[:], plocal[:])
            # transpose to [j, l] so HBM rows are natural S order
            pft = tailpool.tile([NB, P], F32, tag="tail")
            nc.tensor.transpose(pft[:], field_sb[:], ident[:])
            fieldT = mpool.tile([NB, P], F32, tag="fieldT")
            nc.vector.tensor_copy(fieldT[:], pft[:])
            nc.sync.dma_start(field_o[:], fieldT[:])
            gateT = mpool.tile([NB, P], F32, tag="gateT")
            nc.scalar.activation(gateT[:], pft[:], AF.Sigmoid, scale=-float(lam))
            nc.sync.dma_start(gate_o[:], gateT[:])

    nc.compile()
    return nc


def host_constants(S, D, C):
    """EMA / transpose constants, computed in float64 then cast."""
    NB = S // P
    l = np.arange(P)
    # T[l, k] = (1-a) a^(l-k) for k <= l ; lhsT for the matmul is T.T
    T = np.where(l[:, None] >= l[None, :],
                 (1.0 - ALPHA) * ALPHA ** (l[:, None] - l[None, :]), 0.0)
    tlt = np.ascontiguousarray(T.T).astype(np.float32)
    aL = ALPHA ** P
    j = np.arange(NB)
    with np.errstate(under="ignore"):
        t2r = np.where(j[:, None] <= j[None, :] - 1,
                       aL ** (j[None, :] - 1 - j[:, None]), 0.0).astype(np.float32)
    apow = (ALPHA ** (l + 1)).astype(np.float32).reshape(1, P)
    t127 = ((1.0 - ALPHA) * ALPHA ** (P - 1 - l)).astype(np.float32).reshape(P, 1)
    ident = np.eye(P, dtype=np.float32)
    return tlt, t2r, apow, t127, ident


_NC_CACHE = {}


def _get_nc(S, D, C, CHUNK, b2val, lam, use_f32r=True):
    key = (S, D, C, CHUNK, b2val, lam, use_f32r)
    if key not in _NC_CACHE:
        _NC_CACHE[key] = build_nc(S, D, C, CHUNK, b2val, lam, use_f32r)
    return _NC_CACHE[key]


def _run(inputs, trace=False, use_f32r=True, tmpdir=None):
    hidden = np.asarray(inputs["hidden"], dtype=np.float32)
    fiber_w = np.asarray(inputs["fiber_w"], dtype=np.float32)
    w1 = np.asarray(inputs["w1"], dtype=np.float32)
    b1 = np.asarray(inputs["b1"], dtype=np.float32)
    w2 = np.asarray(inputs["w2"], dtype=np.float32)
    b2 = np.asarray(inputs["b2"], dtype=np.float32)
    lam = float(np.asarray(inputs["lambda_gate"]))

    B, S, D = hidden.shape
    C = w1.shape[0]
    NB = S // P
    b2val = float(b2.reshape(-1)[0])

    weff = w1[:, :D] + w1[:, D:] @ fiber_w                    # [C, D]
    weffT = np.ascontiguousarray(weff.T)                      # [D, C]
    # gelu is computed as u*(1+erf(u/sqrt2)) with the 0.5 folded into w2
    w2col = np.ascontiguousarray(0.5 * w2.reshape(1, C).T)    # [C, 1]
    b1col = np.ascontiguousarray(b1.reshape(C, 1))
    b1s = np.ascontiguousarray((b1 / np.sqrt(2.0)).astype(np.float32).reshape(C, 1))
    tlt, t2r, apow, t127, ident = host_constants(S, D, C)

    nc = _get_nc(S, D, C, 512, b2val, lam, use_f32r)

    shared = {
        "weffT": weffT, "w2col": w2col, "b1col": b1col, "b1s": b1s,
        "ident": ident, "tlt": tlt, "t2r": t2r, "apow": apow, "t127": t127,
        "b2col": np.full((P, 1), b2val, dtype=np.float32),
        "ones1": np.ones((1, 1), dtype=np.float32),
    }
    in_maps = [dict(shared, hidden_b=np.ascontiguousarray(hidden[b]))
               for b in range(B)]

    res = run_bass_kernel_spmd(nc, in_maps, core_ids=list(range(N_CORES)),
                               trace=trace, tmpdir=tmpdir)

    gate = np.empty((B, S), dtype=np.float32)
    field = np.empty((B, S), dtype=np.float32)
    rtot = 0.0
    for b in range(B):
        out = res.results[b]
        gate[b] = np.asarray(out["gate_out"]).reshape(S)
        field[b] = np.asarray(out["field_out"]).reshape(S)
        rtot += float(np.asarray(out["rsum_out"], dtype=np.float64).sum())
    risk = np.float32(rtot / (B * S))
    return (gate, field, risk), res.exec_time_ns


def kernel(**inputs):
    out, _ = _run(inputs, trace=False)
    return out


# revision 22
# speedup vs baseline: 1.0062x; 1.0062x over previous
"""Trainium2 Bass kernel for nn_CFHoTWrapper (fiber-adapter + causal EMA + gate).

Contract: kernel(**inputs) takes the FULL unsharded inputs (hidden [8,8192,2048],
adapter weights) and returns (gate [8,8192], field [8,8192], risk scalar),
matching reference.reference(). Work is data-parallel over batch B across the
8 NeuronCores (one batch element per core); the tiny adapter weights are
replicated.

Math notes:
  - fiber = h @ fiber_w.T is linear and feeds straight into the concat+w1
    matmul, so it folds:  combined @ w1.T = h @ (w1[:, :D] + w1[:, D:] @ fiber_w).T
    Host precomputes Weff [C, D] once.
  - The causal EMA over S is a linear recurrence; it is computed as a blocked
    prefix: a [128,128] lower-triangular matmul gives within-block prefixes,
    a [NB,NB] triangular matmul gives cross-block carries, and a rank-1
    outer-product matmul (alpha^(l+1) x carry_j) accumulates the carry terms.
"""

import os
import sys

for _p in ("/opt/trn_rl_repo", "/root/.axon_site/_ro/trn_rl_repo"):
    if os.path.isdir(_p) and _p not in sys.path:
        sys.path.insert(0, _p)

import numpy as np

import concourse.bass as bass
import concourse.mybir as mybir
import concourse.tile as tile
from concourse import bacc
from concourse.tile_rust import add_dep_helper
from concourse.bass_utils import run_bass_kernel_spmd

F32 = mybir.dt.float32
F16 = mybir.dt.float16
F32R = mybir.dt.float32r
AF = mybir.ActivationFunctionType

ALPHA = 0.9
P = 128
N_CORES = 8


def build_nc(S=8192, D=2048, C=64, CHUNK=512, b2val=0.0, lam=1.0, use_f32r=True,
             act="erf"):
    """Build + compile the per-core Bass program (SPMD: identical on all cores).

    Inputs (per core): hidden_b [S, D], weffT [D, C], w2col [C, 1], b1col [C, 1],
    ident [128, 128], tlt [128, 128], t2r [NB, NB], apow [1, 128], ones1 [1, 1].
    Outputs: gate_out [NB, 128], field_out [NB, 128] (row-major == natural S
    order), rsum_out [128, 1] (partial sums of delta for the risk mean).
    """
    NB = S // P           # EMA blocks of 128 tokens
    NCH = S // CHUNK      # token chunks processed per loop iteration
    SPC = CHUNK // P      # 128-token tiles per chunk
    KD = D // P           # contraction blocks
    assert S % CHUNK == 0 and CHUNK % P == 0 and D % P == 0

    nc = bacc.Bacc("TRN2", target_bir_lowering=False, debug=False,
                   enable_asserts=False)

    h_d = nc.dram_tensor("hidden_b", [S, D], F32, kind="ExternalInput").ap()
    weffT_d = nc.dram_tensor("weffT", [D, C], F32, kind="ExternalInput").ap()
    w2c_d = nc.dram_tensor("w2col", [C, 1], F32, kind="ExternalInput").ap()
    b1c_d = nc.dram_tensor("b1col", [C, 1], F32, kind="ExternalInput").ap()
    b1s_d = nc.dram_tensor("b1s", [C, 1], F32, kind="ExternalInput").ap()
    id_d = nc.dram_tensor("ident", [P, P], F32, kind="ExternalInput").ap()
    tlt_d = nc.dram_tensor("tlt", [P, P], F32, kind="ExternalInput").ap()
    t2r_d = nc.dram_tensor("t2r", [NB, NB], F32, kind="ExternalInput").ap()
    apow_d = nc.dram_tensor("apow", [1, P], F32, kind="ExternalInput").ap()
    t127_d = nc.dram_tensor("t127", [P, 1], F32, kind="ExternalInput").ap()
    b2c_d = nc.dram_tensor("b2col", [P, 1], F32, kind="ExternalInput").ap()
    one_d = nc.dram_tensor("ones1", [1, 1], F32, kind="ExternalInput").ap()

    gate_o = nc.dram_tensor("gate_out", [NB, P], F32, kind="ExternalOutput").ap()
    field_o = nc.dram_tensor("field_out", [NB, P], F32, kind="ExternalOutput").ap()
    rsum_o = nc.dram_tensor("rsum_out", [P, 1], F32, kind="ExternalOutput").ap()
    warm_o = nc.dram_tensor("warm_out", [1, 1], F32, kind="ExternalOutput").ap()

    RT = F16 if use_f32r else F32

    with tile.TileContext(nc) as tc:
        with (
            tc.tile_pool(name="consts", bufs=1) as cpool,
            tc.tile_pool(name="hin", bufs=4) as hpool,
            tc.tile_pool(name="hhalf", bufs=2) as hhpool,
            tc.tile_pool(name="ht", bufs=4) as htpool,
            tc.tile_pool(name="hid", bufs=2) as hidpool,
            tc.tile_pool(name="misc", bufs=1) as mpool,
            tc.tile_pool(name="pt", bufs=3, space=bass.MemorySpace.PSUM) as ptpool,
            tc.tile_pool(name="ph", bufs=2, space=bass.MemorySpace.PSUM) as phpool,
            tc.tile_pool(name="pd", bufs=1, space=bass.MemorySpace.PSUM) as pdpool,
            tc.tile_pool(name="ptail", bufs=2, space=bass.MemorySpace.PSUM) as tailpool,
        ):
            weffT = cpool.tile([P, KD, C], F32, tag="weffT")
            nc.sync.dma_start(weffT[:], weffT_d.rearrange("(kd p) c -> p kd c", p=P))
            # cast the stationary weights once on DVE: fp16 weights get
            # FWL (fast weight load) on the PE; fp32/f32r LDWEIGHTS serialize.
            weffT_r = cpool.tile([P, KD, C], RT, tag="weffT_r")
            nc.vector.tensor_copy(weffT_r[:], weffT[:])
            ident = cpool.tile([P, P], F32, tag="ident")
            nc.sync.dma_start(ident[:], id_d[:])
            identr = cpool.tile([P, P], RT, tag="identr")
            nc.vector.tensor_copy(identr[:], ident[:])
            identh = cpool.tile([P, P], F16, tag="identh")
            nc.vector.tensor_copy(identh[:], ident[:])
            tlt = cpool.tile([P, P], F32, tag="tlt")
            nc.sync.dma_start(tlt[:], tlt_d[:])
            t2r = cpool.tile([NB, NB], F32, tag="t2r")
            nc.sync.dma_start(t2r[:], t2r_d[:])
            w2c = cpool.tile([C, 1], F32, tag="w2c")
            nc.sync.dma_start(w2c[:], w2c_d[:])
            w2h = cpool.tile([C, 1], RT, tag="w2h")
            nc.vector.tensor_copy(w2h[:], w2c[:])
            b1c = cpool.tile([C, 1], F32, tag="b1c")
            nc.sync.dma_start(b1c[:], b1c_d[:])
            b1s = cpool.tile([C, 1], F32, tag="b1s")
            nc.sync.dma_start(b1s[:], b1s_d[:])
            apow = cpool.tile([1, P], F32, tag="apow")
            nc.sync.dma_start(apow[:], apow_d[:])
            t127 = cpool.tile([P, 1], F32, tag="t127")
            nc.sync.dma_start(t127[:], t127_d[:])
            b2c = cpool.tile([P, 1], F32, tag="b2c")
            nc.sync.dma_start(b2c[:], b2c_d[:])
            one1 = cpool.tile([1, 1], F32, tag="one1")
            nc.sync.dma_start(one1[:], one_d[:])

            # delta (pre-softplus) accumulates here across the whole sequence,
            # laid out [l, j] with token s = j*128 + l.
            pdelta = pdpool.tile([P, NB], F32, tag="pdelta")

            # HAM warmup: transpose-mode matmuls don't count as PE-busy for
            # the activity monitor, so without real-matmul pressure the PE
            # stays clock-gated at 1.2 GHz. A dense burst of normal matmuls
            # here (overlapping the first chunk's DMA) flips it to 2.4 GHz;
            # afterwards a real matmul lands every ~1us so it never
            # re-throttles (MID window needs ~3.4us of continuous idle).
            pwarm = ptpool.tile([P, CHUNK], F32, tag="pt")
            for _w in range(32):
                nwk = min(KD, CHUNK // C)
                nc.tensor.matmul(pwarm[:, 0:nwk * C], identr[:],
                                 weffT_r[:, 0:nwk, :],
                                 start=True, stop=True)
            warm_sb = mpool.tile([1, 1], F32, tag="warmsb")
            nc.vector.tensor_copy(warm_sb[:], pwarm[0:1, 0:1])
            nc.sync.dma_start(warm_o[:], warm_sb[:])

            h_view = h_d.rearrange("(ch sp p) d -> ch p sp d", sp=SPC, p=P)
            dma_chain = []
            for ch in range(NCH):
                htile = hpool.tile([P, SPC, D], F32, tag="hin")
                # Issue each chunk as two half-DMAs on the SP ring, chained so
                # at most 2 halves are in flight. The SDMA engines round-robin
                # across ALL in-flight DMAs, so unbounded prefetch makes chunk
                # 0 finish ~4x late (a ~40us pipeline-fill bubble). The chain
                # keeps completion order == processing order while the ring
                # stays continuously busy.
                half = SPC // 2
                for hf in range(2):
                    dins = nc.sync.dma_start(
                        htile[:, hf * half:(hf + 1) * half, :],
                        h_view[ch][:, hf * half:(hf + 1) * half, :])
                    dma_chain.append(dins.ins)
                    if len(dma_chain) > 2:
                        add_dep_helper(dma_chain[-1], dma_chain[-3],
                                       reason="cap in-flight input DMAs at 2")
                # cast h to fp16 once per s-tile: fp16 transposes stream at
                # 1 cyc/row (vs 2 for fp32) and h gets rounded to 11 mantissa
                # bits anyway by the fp32r matmul (12 bits) - accuracy cost is
                # ~1.2x, PE cost halves. Cast rides on DVE/ACT headroom.
                hhalf = hhpool.tile([P, SPC, D], F16, tag="hhalf")
                for sp in range(SPC):
                    if sp < SPC - 1:
                        nc.vector.tensor_copy(hhalf[:, sp, :], htile[:, sp, :])
                    else:
                        nc.scalar.copy(hhalf[:, sp, :], htile[:, sp, :])
                phid = phpool.tile([C, CHUNK], F32, tag="phid")
                # software-pipelined by one kd: the matmul for kd-1 is emitted
                # after the transposes for kd, so the PE never stalls on the
                # PSUM->SBUF copy latency.
                prev_ht = None
                for kd in range(KD):
                    pt = ptpool.tile([P, CHUNK], F16, tag="pt")
                    for sp in range(SPC):
                        nc.tensor.transpose(
                            pt[:, sp * P:(sp + 1) * P],
                            hhalf[:, sp, kd * P:(kd + 1) * P],
                            identh[:],
                        )
                    ht = htpool.tile([P, CHUNK], RT, tag="ht")
                    if kd % 2 == 0:
                        nc.vector.tensor_copy(ht[:], pt[:])
                    else:
                        nc.scalar.copy(ht[:], pt[:])
                    if prev_ht is not None:
                        nc.tensor.matmul(
                            phid[:], weffT_r[:, kd - 1, :], prev_ht[:],
                            start=(kd == 1), stop=False)
                    prev_ht = ht
                nc.tensor.matmul(
                    phid[:], weffT_r[:, KD - 1, :], prev_ht[:],
                    start=(KD == 1), stop=True)
                # exact gelu: u*0.5*(1+erf(u/sqrt2)); the 0.5 is folded into
                # w2col on the host. b1c is pre-scaled by 1/sqrt2 for the erf
                # input (b1s), raw for the u copy (b1c).
                # CoreSim has no Erf; act="tanh" swaps it for sim validation.
                afn = AF.Erf if act == "erf" else AF.Tanh
                erf_t = hidpool.tile([C, CHUNK], F32, tag="erf_t")
                nc.scalar.activation(erf_t[:], phid[:], afn,
                                     scale=float(1.0 / np.sqrt(2.0)), bias=b1s[:])
                u_sb = hidpool.tile([C, CHUNK], F32, tag="u_sb")
                nc.scalar.activation(u_sb[:], phid[:], AF.Identity, bias=b1c[:])
                hidT = hidpool.tile([C, CHUNK], RT, tag="hidT")
                nc.vector.scalar_tensor_tensor(
                    hidT[:], in0=erf_t[:], scalar=1.0, in1=u_sb[:],
                    op0=mybir.AluOpType.add, op1=mybir.AluOpType.mult)
                for b in range(SPC):
                    j = ch * SPC + b
                    nc.tensor.matmul(
                        pdelta[:, j:j + 1],
                        hidT[:, b * P:(b + 1) * P],
                        w2h[:],
                        start=True,
                        stop=True,
                    )

            # delta = softplus(. + b2) = ln(1 + exp(. + b2)) — Softplus has no
            # loadable ACT table here, but Exp and Ln share one set.
            # accum_out on the Ln gives per-partition delta sums for risk.
            texp = mpool.tile([P, NB], F32, tag="texp")
            nc.scalar.activation(texp[:], pdelta[:], AF.Exp, bias=b2c[:])
            t1p = mpool.tile([P, NB], F32, tag="t1p")
            nc.vector.tensor_scalar_add(t1p[:], texp[:], 1.0)
            delta_sb = mpool.tile([P, NB], F32, tag="delta")
            rsum = mpool.tile([P, 1], F32, tag="rsum")
            nc.scalar.activation(delta_sb[:], t1p[:], AF.Ln, accum_out=rsum[:])
            nc.sync.dma_start(rsum_o[:], rsum[:])

            # EMA: within-block prefix via triangular matmul
            plocal = tailpool.tile([P, NB], F32, tag="tail")
            nc.tensor.matmul(plocal[:], tlt[:], delta_sb[:], start=True, stop=True)
            # last row of each block (PSUM can't be read at partition 127;
            # recompute it from delta_sb with the T's last row) -> [1, NB]
            pllr = tailpool.tile([1, NB], F32, tag="tail")
            nc.tensor.matmul(pllr[:], t127[:], delta_sb[:], start=True, stop=True)
            ll_row = mpool.tile([1, NB], F32, tag="llrow")
            nc.vector.tensor_copy(ll_row[:], pllr[:])
            pllc = tailpool.tile([NB, 1], F32, tag="tail")
            nc.tensor.matmul(pllc[:], ll_row[:], one1[:], start=True, stop=True)
            llc = mpool.tile([NB, 1], F32, tag="llc")
            nc.vector.tensor_copy(llc[:], pllc[:])
            # carries into each block: C[j] = sum_i (alpha^128)^(j-1-i) ll[i], i<=j-1
            pcr = tailpool.tile([1, NB], F32, tag="tail")
            nc.tensor.matmul(pcr[:], llc[:], t2r[:], start=True, stop=True)
            crow = mpool.tile([1, NB], F32, tag="crow")
            nc.vector.tensor_copy(crow[:], pcr[:])
            # field[l, j] = local[l, j] + alpha^(l+1) * C[j]
            nc.tensor.matmul(plocal[:], apow[:], crow[:], start=False, stop=True,
                             skip_group_check=True)
            field_sb = mpool.tile([P, NB], F32, tag="fieldsb")
            nc.vector.tensor_copy(field_sb[:], plocal[:])
            # transpose to [j, l] so HBM rows are natural S order
            pft = tailpool.tile([NB, P], F32, tag="tail")
            nc.tensor.transpose(pft[:], field_sb[:], ident[:])
            fieldT = mpool.tile([NB, P], F32, tag="fieldT")
            nc.vector.tensor_copy(fieldT[:], pft[:])
            nc.sync.dma_start(field_o[:], fieldT[:])
            gateT = mpool.tile([NB, P], F32, tag="gateT")
            nc.scalar.activation(gateT[:], pft[:], AF.Sigmoid, scale=-float(lam))
            nc.sync.dma_start(gate_o[:], gateT[:])

    nc.compile()
    return nc


def host_constants(S, D, C):
    """EMA / transpose constants, computed in float64 then cast."""
    NB = S // P
    l = np.arange(P)
    # T[l, k] = (1-a) a^(l-k) for k <= l ; lhsT for the matmul is T.T
    T = np.where(l[:, None] >= l[None, :],
                 (1.0 - ALPHA) * ALPHA ** (l[:, None] - l[None, :]), 0.0)
    tlt = np.ascontiguousarray(T.T).astype(np.float32)
    aL = ALPHA ** P
    j = np.arange(NB)
    with np.errstate(under="ignore"):
        t2r = np.where(j[:, None] <= j[None, :] - 1,
                       aL ** (j[None, :] - 1 - j[:, None]), 0.0).astype(np.float32)
    apow = (ALPHA ** (l + 1)).astype(np.float32).reshape(1, P)
    t127 = ((1.0 - ALPHA) * ALPHA ** (P - 1 - l)).astype(np.float32).reshape(P, 1)
    ident = np.eye(P, dtype=np.float32)
    return tlt, t2r, apow, t127, ident


_NC_CACHE = {}


def _get_nc(S, D, C, CHUNK, b2val, lam, use_f32r=True):
    key = (S, D, C, CHUNK, b2val, lam, use_f32r)
    if key not in _NC_CACHE:
        _NC_CACHE[key] = build_nc(S, D, C, CHUNK, b2val, lam, use_f32r)
    return _NC_CACHE[key]


def _run(inputs, trace=False, use_f32r=True, tmpdir=None):
    hidden = np.asarray(inputs["hidden"], dtype=np.float32)
    fiber_w = np.asarray(inputs["fiber_w"], dtype=np.float32)
    w1 = np.asarray(inputs["w1"], dtype=np.float32)
    b1 = np.asarray(inputs["b1"], dtype=np.float32)
    w2 = np.asarray(inputs["w2"], dtype=np.float32)
    b2 = np.asarray(inputs["b2"], dtype=np.float32)
    lam = float(np.asarray(inputs["lambda_gate"]))

    B, S, D = hidden.shape
    C = w1.shape[0]
    NB = S // P
    b2val = float(b2.reshape(-1)[0])

    weff = w1[:, :D] + w1[:, D:] @ fiber_w                    # [C, D]
    weffT = np.ascontiguousarray(weff.T)                      # [D, C]
    # gelu is computed as u*(1+erf(u/sqrt2)) with the 0.5 folded into w2
    w2col = np.ascontiguousarray(0.5 * w2.reshape(1, C).T)    # [C, 1]
    b1col = np.ascontiguousarray(b1.reshape(C, 1))
    b1s = np.ascontiguousarray((b1 / np.sqrt(2.0)).astype(np.float32).reshape(C, 1))
    tlt, t2r, apow, t127, ident = host_constants(S, D, C)

    nc = _get_nc(S, D, C, 512, b2val, lam, use_f32r)

    shared = {
        "weffT": weffT, "w2col": w2col, "b1col": b1col, "b1s": b1s,
        "ident": ident, "tlt": tlt, "t2r": t2r, "apow": apow, "t127": t127,
        "b2col": np.full((P, 1), b2val, dtype=np.float32),
        "ones1": np.ones((1, 1), dtype=np.float32),
    }
    in_maps = [dict(shared, hidden_b=np.ascontiguousarray(hidden[b]))
               for b in range(B)]

    res = run_bass_kernel_spmd(nc, in_maps, core_ids=list(range(N_CORES)),
                               trace=trace, tmpdir=tmpdir)

    gate = np.empty((B, S), dtype=np.float32)
    field = np.empty((B, S), dtype=np.float32)
    rtot = 0.0
    for b in range(B):
        out = res.results[b]
        gate[b] = np.asarray(out["gate_out"]).reshape(S)
        field[b] = np.asarray(out["field_out"]).reshape(S)
        rtot += float(np.asarray(out["rsum_out"], dtype=np.float64).sum())
    risk = np.float32(rtot / (B * S))
    return (gate, field, risk), res.exec_time_ns


def kernel(**inputs):
    out, _ = _run(inputs, trace=False)
    return out


# revision 23
# speedup vs baseline: 1.1542x; 1.1470x over previous
"""Trainium2 Bass kernel for nn_CFHoTWrapper (fiber-adapter + causal EMA + gate).

Contract: kernel(**inputs) takes the FULL unsharded inputs (hidden [8,8192,2048],
adapter weights) and returns (gate [8,8192], field [8,8192], risk scalar),
matching reference.reference(). Work is data-parallel over batch B across the
8 NeuronCores (one batch element per core); the tiny adapter weights are
replicated.

Math notes:
  - fiber = h @ fiber_w.T is linear and feeds straight into the concat+w1
    matmul, so it folds:  combined @ w1.T = h @ (w1[:, :D] + w1[:, D:] @ fiber_w).T
    Host precomputes Weff [C, D] once.
  - The causal EMA over S is a linear recurrence; it is computed as a blocked
    prefix: a [128,128] lower-triangular matmul gives within-block prefixes,
    a [NB,NB] triangular matmul gives cross-block carries, and a rank-1
    outer-product matmul (alpha^(l+1) x carry_j) accumulates the carry terms.
"""

import os
import sys

for _p in ("/opt/trn_rl_repo", "/root/.axon_site/_ro/trn_rl_repo"):
    if os.path.isdir(_p) and _p not in sys.path:
        sys.path.insert(0, _p)

import numpy as np

import concourse.bass as bass
import concourse.mybir as mybir
import concourse.tile as tile
from concourse import bacc
from concourse.tile_rust import add_dep_helper
from concourse.bass_utils import run_bass_kernel_spmd

F32 = mybir.dt.float32
F16 = mybir.dt.float16
F32R = mybir.dt.float32r
AF = mybir.ActivationFunctionType

ALPHA = 0.9
P = 128
N_CORES = 8


def build_nc(S=8192, D=2048, C=64, CHUNK=512, b2val=0.0, lam=1.0, use_f32r=True,
             act="erf"):
    """Build + compile the per-core Bass program (SPMD: identical on all cores).

    Inputs (per core): hidden_b [S, D], weffT [D, C], w2col [C, 1], b1col [C, 1],
    ident [128, 128], tlt [128, 128], t2r [NB, NB], apow [1, 128], ones1 [1, 1].
    Outputs: gate_out [NB, 128], field_out [NB, 128] (row-major == natural S
    order), rsum_out [128, 1] (partial sums of delta for the risk mean).
    """
    NB = S // P           # EMA blocks of 128 tokens
    NCH = S // CHUNK      # token chunks processed per loop iteration
    SPC = CHUNK // P      # 128-token tiles per chunk
    KD = D // P           # contraction blocks
    assert S % CHUNK == 0 and CHUNK % P == 0 and D % P == 0

    nc = bacc.Bacc("TRN2", target_bir_lowering=False, debug=False,
                   enable_asserts=False)

    h_d = nc.dram_tensor("hidden_b", [S, D], F32, kind="ExternalInput").ap()
    weffT_d = nc.dram_tensor("weffT", [D, C], F32, kind="ExternalInput").ap()
    w2c_d = nc.dram_tensor("w2col", [C, 1], F32, kind="ExternalInput").ap()
    b1c_d = nc.dram_tensor("b1col", [C, 1], F32, kind="ExternalInput").ap()
    b1s_d = nc.dram_tensor("b1s", [C, 1], F32, kind="ExternalInput").ap()
    id_d = nc.dram_tensor("ident", [P, P], F32, kind="ExternalInput").ap()
    tlt_d = nc.dram_tensor("tlt", [P, P], F32, kind="ExternalInput").ap()
    t2r_d = nc.dram_tensor("t2r", [NB, NB], F32, kind="ExternalInput").ap()
    apow_d = nc.dram_tensor("apow", [1, P], F32, kind="ExternalInput").ap()
    t127_d = nc.dram_tensor("t127", [P, 1], F32, kind="ExternalInput").ap()
    b2c_d = nc.dram_tensor("b2col", [P, 1], F32, kind="ExternalInput").ap()
    one_d = nc.dram_tensor("ones1", [1, 1], F32, kind="ExternalInput").ap()

    gate_o = nc.dram_tensor("gate_out", [NB, P], F32, kind="ExternalOutput").ap()
    field_o = nc.dram_tensor("field_out", [NB, P], F32, kind="ExternalOutput").ap()
    rsum_o = nc.dram_tensor("rsum_out", [P, 1], F32, kind="ExternalOutput").ap()
    warm_o = nc.dram_tensor("warm_out", [1, 1], F32, kind="ExternalOutput").ap()

    RT = F16 if use_f32r else F32

    with tile.TileContext(nc) as tc:
        with (
            tc.tile_pool(name="consts", bufs=1) as cpool,
            tc.tile_pool(name="hin", bufs=4) as hpool,
            tc.tile_pool(name="hhalf", bufs=2) as hhpool,
            tc.tile_pool(name="ht", bufs=4) as htpool,
            tc.tile_pool(name="hid", bufs=2) as hidpool,
            tc.tile_pool(name="misc", bufs=1) as mpool,
            tc.tile_pool(name="pt", bufs=3, space=bass.MemorySpace.PSUM) as ptpool,
            tc.tile_pool(name="ph", bufs=2, space=bass.MemorySpace.PSUM) as phpool,
            tc.tile_pool(name="pd", bufs=1, space=bass.MemorySpace.PSUM) as pdpool,
            tc.tile_pool(name="ptail", bufs=2, space=bass.MemorySpace.PSUM) as tailpool,
        ):
            weffT = cpool.tile([P, KD, C], F32, tag="weffT")
            nc.sync.dma_start(weffT[:], weffT_d.rearrange("(kd p) c -> p kd c", p=P))
            # cast the stationary weights once on DVE: fp16 weights get
            # FWL (fast weight load) on the PE; fp32/f32r LDWEIGHTS serialize.
            weffT_r = cpool.tile([P, KD, C], RT, tag="weffT_r")
            nc.vector.tensor_copy(weffT_r[:], weffT[:])
            ident = cpool.tile([P, P], F32, tag="ident")
            nc.sync.dma_start(ident[:], id_d[:])
            identr = cpool.tile([P, P], RT, tag="identr")
            nc.vector.tensor_copy(identr[:], ident[:])
            identh = cpool.tile([P, P], F16, tag="identh")
            nc.vector.tensor_copy(identh[:], ident[:])
            tlt = cpool.tile([P, P], F32, tag="tlt")
            nc.sync.dma_start(tlt[:], tlt_d[:])
            t2r = cpool.tile([NB, NB], F32, tag="t2r")
            nc.sync.dma_start(t2r[:], t2r_d[:])
            w2c = cpool.tile([C, 1], F32, tag="w2c")
            nc.sync.dma_start(w2c[:], w2c_d[:])
            w2h = cpool.tile([C, 1], RT, tag="w2h")
            nc.vector.tensor_copy(w2h[:], w2c[:])
            b1c = cpool.tile([C, 1], F32, tag="b1c")
            nc.sync.dma_start(b1c[:], b1c_d[:])
            b1s = cpool.tile([C, 1], F32, tag="b1s")
            nc.sync.dma_start(b1s[:], b1s_d[:])
            apow = cpool.tile([1, P], F32, tag="apow")
            nc.sync.dma_start(apow[:], apow_d[:])
            t127 = cpool.tile([P, 1], F32, tag="t127")
            nc.sync.dma_start(t127[:], t127_d[:])
            b2c = cpool.tile([P, 1], F32, tag="b2c")
            nc.sync.dma_start(b2c[:], b2c_d[:])
            one1 = cpool.tile([1, 1], F32, tag="one1")
            nc.sync.dma_start(one1[:], one_d[:])

            # delta (pre-softplus) accumulates here across the whole sequence,
            # laid out [l, j] with token s = j*128 + l.
            pdelta = pdpool.tile([P, NB], F32, tag="pdelta")

            # HAM warmup: transpose-mode matmuls don't count as PE-busy for
            # the activity monitor, so without real-matmul pressure the PE
            # stays clock-gated at 1.2 GHz. A dense burst of normal matmuls
            # here (overlapping the first chunk's DMA) flips it to 2.4 GHz;
            # afterwards a real matmul lands every ~1us so it never
            # re-throttles (MID window needs ~3.4us of continuous idle).
            pwarm = ptpool.tile([P, CHUNK], F32, tag="pt")
            for _w in range(32):
                nwk = min(KD, CHUNK // C)
                nc.tensor.matmul(pwarm[:, 0:nwk * C], identr[:],
                                 weffT_r[:, 0:nwk, :],
                                 start=True, stop=True)
            warm_sb = mpool.tile([1, 1], F32, tag="warmsb")
            nc.vector.tensor_copy(warm_sb[:], pwarm[0:1, 0:1])
            nc.sync.dma_start(warm_o[:], warm_sb[:])

            h_view = h_d.rearrange("(ch sp p) d -> ch p sp d", sp=SPC, p=P)
            dma_chain = []
            for ch in range(NCH):
                htile = hpool.tile([P, SPC, D], F32, tag="hin")
                # alternate HWDGE rings (SP / ACT) for the bulk input stream
                (nc.sync if ch % 2 == 0 else nc.scalar).dma_start(
                    htile[:], h_view[ch])
                # cast h to fp16 once per s-tile: fp16 transposes stream at
                # 1 cyc/row (vs 2 for fp32) and h gets rounded to 11 mantissa
                # bits anyway by the fp32r matmul (12 bits) - accuracy cost is
                # ~1.2x, PE cost halves. Cast rides on DVE/ACT headroom.
                hhalf = hhpool.tile([P, SPC, D], F16, tag="hhalf")
                for sp in range(SPC):
                    if sp < SPC - 1:
                        nc.vector.tensor_copy(hhalf[:, sp, :], htile[:, sp, :])
                    else:
                        nc.scalar.copy(hhalf[:, sp, :], htile[:, sp, :])
                phid = phpool.tile([C, CHUNK], F32, tag="phid")
                # software-pipelined by one kd: the matmul for kd-1 is emitted
                # after the transposes for kd, so the PE never stalls on the
                # PSUM->SBUF copy latency.
                prev_ht = None
                for kd in range(KD):
                    pt = ptpool.tile([P, CHUNK], F16, tag="pt")
                    for sp in range(SPC):
                        nc.tensor.transpose(
                            pt[:, sp * P:(sp + 1) * P],
                            hhalf[:, sp, kd * P:(kd + 1) * P],
                            identh[:],
                        )
                    ht = htpool.tile([P, CHUNK], RT, tag="ht")
                    if kd % 2 == 0:
                        nc.vector.tensor_copy(ht[:], pt[:])
                    else:
                        nc.scalar.copy(ht[:], pt[:])
                    if prev_ht is not None:
                        nc.tensor.matmul(
                            phid[:], weffT_r[:, kd - 1, :], prev_ht[:],
                            start=(kd == 1), stop=False)
                    prev_ht = ht
                nc.tensor.matmul(
                    phid[:], weffT_r[:, KD - 1, :], prev_ht[:],
                    start=(KD == 1), stop=True)
                # exact gelu: u*0.5*(1+erf(u/sqrt2)); the 0.5 is folded into
                # w2col on the host. b1c is pre-scaled by 1/sqrt2 for the erf
                # input (b1s), raw for the u copy (b1c).
                # CoreSim has no Erf; act="tanh" swaps it for sim validation.
                afn = AF.Erf if act == "erf" else AF.Tanh
                erf_t = hidpool.tile([C, CHUNK], F32, tag="erf_t")
                nc.scalar.activation(erf_t[:], phid[:], afn,
                                     scale=float(1.0 / np.sqrt(2.0)), bias=b1s[:])
                u_sb = hidpool.tile([C, CHUNK], F32, tag="u_sb")
                nc.scalar.activation(u_sb[:], phid[:], AF.Identity, bias=b1c[:])
                hidT = hidpool.tile([C, CHUNK], RT, tag="hidT")
                nc.vector.scalar_tensor_tensor(
                    hidT[:], in0=erf_t[:], scalar=1.0, in1=u_sb[:],
                    op0=mybir.AluOpType.add, op1=mybir.AluOpType.mult)
                for b in range(SPC):
                    j = ch * SPC + b
                    nc.tensor.matmul(
                        pdelta[:, j:j + 1],
                        hidT[:, b * P:(b + 1) * P],
                        w2h[:],
                        start=True,
                        stop=True,
                    )

            # delta = softplus(. + b2) = ln(1 + exp(. + b2)) — Softplus has no
            # loadable ACT table here, but Exp and Ln share one set.
            # accum_out on the Ln gives per-partition delta sums for risk.
            texp = mpool.tile([P, NB], F32, tag="texp")
            nc.scalar.activation(texp[:], pdelta[:], AF.Exp, bias=b2c[:])
            t1p = mpool.tile([P, NB], F32, tag="t1p")
            nc.vector.tensor_scalar_add(t1p[:], texp[:], 1.0)
            delta_sb = mpool.tile([P, NB], F32, tag="delta")
            rsum = mpool.tile([P, 1], F32, tag="rsum")
            nc.scalar.activation(delta_sb[:], t1p[:], AF.Ln, accum_out=rsum[:])
            nc.sync.dma_start(rsum_o[:], rsum[:])

            # EMA: within-block prefix via triangular matmul
            plocal = tailpool.tile([P, NB], F32, tag="tail")
            nc.tensor.matmul(plocal[:], tlt[:], delta_sb[:], start=True, stop=True)
            # last row of each block (PSUM can't be read at partition 127;
            # recompute it from delta_sb with the T's last row) -> [1, NB]
            pllr = tailpool.tile([1, NB], F32, tag="tail")
            nc.tensor.matmul(pllr[:], t127[:], delta_sb[:], start=True, stop=True)
            ll_row = mpool.tile([1, NB], F32, tag="llrow")
            nc.vector.tensor_copy(ll_row[:], pllr[:])
            pllc = tailpool.tile([NB, 1], F32, tag="tail")
            nc.tensor.matmul(pllc[:], ll_row[:], one1[:], start=True, stop=True)
            llc = mpool.tile([NB, 1], F32, tag="llc")
            nc.vector.tensor_copy(llc[:], pllc[:])
            # carries into each block: C[j] = sum_i (alpha^128)^(j-1-i) ll[i], i<=j-1
            pcr = tailpool.tile([1, NB], F32, tag="tail")
            nc.tensor.matmul(pcr[:], llc[:], t2r[:], start=True, stop=True)
            crow = mpool.tile([1, NB], F32, tag="crow")
            nc.vector.tensor_copy(crow[:], pcr[:])
            # field[l, j] = local[l, j] + alpha^(l+1) * C[j]
            nc.tensor.matmul(plocal[:], apow[:], crow[:], start=False, stop=True,
                             skip_group_check=True)
            field_sb = mpool.tile([P, NB], F32, tag="fieldsb")
            nc.vector.tensor_copy(field_sb[:], plocal[:])
            # transpose to [j, l] so HBM rows are natural S order
            pft = tailpool.tile([NB, P], F32, tag="tail")
            nc.tensor.transpose(pft[:], field_sb[:], ident[:])
            fieldT = mpool.tile([NB, P], F32, tag="fieldT")
            nc.vector.tensor_copy(fieldT[:], pft[:])
            nc.sync.dma_start(field_o[:], fieldT[:])
            gateT = mpool.tile([NB, P], F32, tag="gateT")
            nc.scalar.activation(gateT[:], pft[:], AF.Sigmoid, scale=-float(lam))
            nc.sync.dma_start(gate_o[:], gateT[:])

    nc.compile()
    return nc


def host_constants(S, D, C):
    """EMA / transpose constants, computed in float64 then cast."""
    NB = S // P
    l = np.arange(P)
    # T[l, k] = (1-a) a^(l-k) for k <= l ; lhsT for the matmul is T.T
    T = np.where(l[:, None] >= l[None, :],
                 (1.0 - ALPHA) * ALPHA ** (l[:, None] - l[None, :]), 0.0)
    tlt = np.ascontiguousarray(T.T).astype(np.float32)
    aL = ALPHA ** P
    j = np.arange(NB)
    with np.errstate(under="ignore"):
        t2r = np.where(j[:, None] <= j[None, :] - 1,
                       aL ** (j[None, :] - 1 - j[:, None]), 0.0).astype(np.float32)
    apow = (ALPHA ** (l + 1)).astype(np.float32).reshape(1, P)
    t127 = ((1.0 - ALPHA) * ALPHA ** (P - 1 - l)).astype(np.float32).reshape(P, 1)
    ident = np.eye(P, dtype=np.float32)
    return tlt, t2r, apow, t127, ident


_NC_CACHE = {}


def _get_nc(S, D, C, CHUNK, b2val, lam, use_f32r=True):
    key = (S, D, C, CHUNK, b2val, lam, use_f32r)
    if key not in _NC_CACHE:
        _NC_CACHE[key] = build_nc(S, D, C, CHUNK, b2val, lam, use_f32r)
    return _NC_CACHE[key]


def _run(inputs, trace=False, use_f32r=True, tmpdir=None):
    hidden = np.asarray(inputs["hidden"], dtype=np.float32)
    fiber_w = np.asarray(inputs["fiber_w"], dtype=np.float32)
    w1 = np.asarray(inputs["w1"], dtype=np.float32)
    b1 = np.asarray(inputs["b1"], dtype=np.float32)
    w2 = np.asarray(inputs["w2"], dtype=np.float32)
    b2 = np.asarray(inputs["b2"], dtype=np.float32)
    lam = float(np.asarray(inputs["lambda_gate"]))

    B, S, D = hidden.shape
    C = w1.shape[0]
    NB = S // P
    b2val = float(b2.reshape(-1)[0])

    weff = w1[:, :D] + w1[:, D:] @ fiber_w                    # [C, D]
    weffT = np.ascontiguousarray(weff.T)                      # [D, C]
    # gelu is computed as u*(1+erf(u/sqrt2)) with the 0.5 folded into w2
    w2col = np.ascontiguousarray(0.5 * w2.reshape(1, C).T)    # [C, 1]
    b1col = np.ascontiguousarray(b1.reshape(C, 1))
    b1s = np.ascontiguousarray((b1 / np.sqrt(2.0)).astype(np.float32).reshape(C, 1))
    tlt, t2r, apow, t127, ident = host_constants(S, D, C)

    nc = _get_nc(S, D, C, 512, b2val, lam, use_f32r)

    shared = {
        "weffT": weffT, "w2col": w2col, "b1col": b1col, "b1s": b1s,
        "ident": ident, "tlt": tlt, "t2r": t2r, "apow": apow, "t127": t127,
        "b2col": np.full((P, 1), b2val, dtype=np.float32),
        "ones1": np.ones((1, 1), dtype=np.float32),
    }
    in_maps = [dict(shared, hidden_b=np.ascontiguousarray(hidden[b]))
               for b in range(B)]

    res = run_bass_kernel_spmd(nc, in_maps, core_ids=list(range(N_CORES)),
                               trace=trace, tmpdir=tmpdir)

    gate = np.empty((B, S), dtype=np.float32)
    field = np.empty((B, S), dtype=np.float32)
    rtot = 0.0
    for b in range(B):
        out = res.results[b]
        gate[b] = np.asarray(out["gate_out"]).reshape(S)
        field[b] = np.asarray(out["field_out"]).reshape(S)
        rtot += float(np.asarray(out["rsum_out"], dtype=np.float64).sum())
    risk = np.float32(rtot / (B * S))
    return (gate, field, risk), res.exec_time_ns


def kernel(**inputs):
    out, _ = _run(inputs, trace=False)
    return out
